# revision 26
# baseline (speedup 1.0000x reference)
"""Multi-head self-attention on 8 Trainium2 NeuronCores (Bass/Tile).

Problem: x[2,2048,1024] -> MHA(16 heads, d_head 64) -> out[2,2048,1024].

Sharding (batch x head-group, Megatron-ish, collective-free):
  core c (0..7): batch b = c//4, head group g = c%4 (heads 4g..4g+3).
  Each core computes q/k/v projections for its 4 heads over its batch,
  attention for those heads, and a PARTIAL output projection
  attn_local[256ch] @ w_out[256ch rows] over the full sequence. The host
  sums the 4 partials per batch (the Megatron row-parallel all-reduce is
  folded into the unshard step; b_out/4 is added on each core so the sum
  carries the bias exactly).

On-core layout (TensorE compute in bf16, fp32 PSUM accumulation):
  - x arrives pre-transposed (x^T, bf16) from the host shard-prep, so the
    kernel does no transposes at all (the xbar serializes against every
    other DMA and was the startup bottleneck when used).
  - qT/kT in [channel, t] layout (weight-stationary matmuls): scores^T =
    kT.T @ qT needs no transposes, and the two heads of a 128-channel chunk
    sit in partitions 0-63/64-127 so their K=64 score matmuls run
    concurrently in disjoint PE row groups.
  - softmax: scores^T [128ki, qi] tiles -> ACT exp (PSUM->SBUF bf16,
    scale=1/8 folded, no max subtraction: |s|/8 <= ~2).
  - PV: attn^T = Vext.T @ P~ with Vext = [V | ones] (M=65): the ones column
    accumulates the softmax denominators in partition 64 for free.
  - normalize: reciprocal_approx_fast + K=1 ones-matmul partition-broadcast,
    software-pipelined one round behind PV so the PE queue never stalls on
    the DVE chain; out-projection chunks follow per query-group.
"""

import numpy as np
import ml_dtypes

import concourse.bass as bass
import concourse.mybir as mybir
import concourse.tile as tile
from concourse import bacc
from concourse import bass_utils
from concourse.bass import ts

BF = mybir.dt.bfloat16
F32 = mybir.dt.float32

B, T, C = 2, 2048, 1024
H, DH = 16, 64
N_CORES = 8
HG = 4  # heads per core
CH = HG * DH  # 256 channels per core

LAST_RESULT = None  # BassKernelResults of the most recent run (for profiling)
_NC_CACHE = None


def _build_nc():
    nc = bacc.Bacc(
        "TRN2", target_bir_lowering=False, debug=False, num_devices=N_CORES
    )

    xt = nc.dram_tensor("xt", [C, T], BF, kind="ExternalInput")
    wq = nc.dram_tensor("wq", [C, CH], BF, kind="ExternalInput")
    wk = nc.dram_tensor("wk", [C, CH], BF, kind="ExternalInput")
    wv = nc.dram_tensor("wv", [C, CH], BF, kind="ExternalInput")
    bqt = nc.dram_tensor("bqt", [128, 2], F32, kind="ExternalInput")
    bkt = nc.dram_tensor("bkt", [128, 2], F32, kind="ExternalInput")
    bv = nc.dram_tensor("bv", [1, CH], F32, kind="ExternalInput")
    wout = nc.dram_tensor("wout", [CH, C], BF, kind="ExternalInput")
    out = nc.dram_tensor("out", [T, C], F32, kind="ExternalOutput")

    with tile.TileContext(nc) as tc:
        with (
            tc.tile_pool(name="persist", bufs=1) as persist,
            tc.tile_pool(name="consts", bufs=1) as consts,
            tc.tile_pool(name="sbn", bufs=6) as sbn,
            tc.tile_pool(name="osb", bufs=3) as osb,
            tc.tile_pool(name="ps_st", bufs=2, space="PSUM") as ps_st,
            tc.tile_pool(name="ps_pv", bufs=2, space="PSUM") as ps_pv,
            tc.tile_pool(name="ps_misc", bufs=2, space="PSUM") as ps_misc,
        ):
            # ---- x^T + weights, one HWDGE queue, dependency-ordered ----
            ones_bf = consts.tile([1, 128], BF)
            nc.vector.memset(ones_bf[:], 1.0)
            ones_col = consts.tile([128, 1], BF)
            nc.vector.memset(ones_col[:], 1.0)
            ones_f32 = consts.tile([1, 128], F32)
            nc.vector.memset(ones_f32[:], 1.0)

            xT = persist.tile([128, 8, T], BF, tag="xT")
            wq_sb = persist.tile([128, 8, CH], BF, tag="wq")
            wk_sb = persist.tile([128, 8, CH], BF, tag="wk")
            wv_sb = persist.tile([128, 8, CH], BF, tag="wv")
            wout_sb = persist.tile([128, 2, C], BF, tag="wout")
            bqt_sb = consts.tile([128, 2], F32)
            bkt_sb = consts.tile([128, 2], F32)
            bv_sb = consts.tile([1, CH], F32)

            xt_r = xt.rearrange("(ci p) t -> p ci t", p=128)
            nc.sync.dma_start(
                out=wq_sb[:], in_=wq.rearrange("(ci p) j -> p ci j", p=128)
            )
            nc.sync.dma_start(
                out=wk_sb[:], in_=wk.rearrange("(ci p) j -> p ci j", p=128)
            )
            nc.sync.dma_start(out=xT[:, 0, :], in_=xt_r[:, 0, :])
            nc.sync.dma_start(out=bqt_sb[:], in_=bqt[:])
            nc.sync.dma_start(out=bkt_sb[:], in_=bkt[:])
            nc.sync.dma_start(out=xT[:, 1, :], in_=xt_r[:, 1, :])
            nc.sync.dma_start(
                out=wv_sb[:], in_=wv.rearrange("(ci p) j -> p ci j", p=128)
            )
            nc.sync.dma_start(out=bv_sb[:], in_=bv[:])
            for ci in range(2, 8):
                nc.sync.dma_start(out=xT[:, ci, :], in_=xt_r[:, ci, :])
            nc.sync.dma_start(
                out=wout_sb[:], in_=wout.rearrange("(hp p) j -> p hp j", p=128)
            )

            bv_rep = persist.tile([128, CH], F32, tag="bv_rep")

            # ---- persistent activations ----
            # qkT[:, 0:2, :] = qT chunks (hp), [:, 2:4, :] = kT chunks;
            # chunk hp rows 0-63 = head 2hp, rows 64-127 = head 2hp+1.
            qkT = persist.tile([128, 4, T], BF, tag="qkT")
            vext = persist.tile([128, T // 128, HG, DH], BF, tag="vext")
            attn_p = [
                [
                    persist.tile(
                        [128, 512], BF, tag=f"attnp{hp}_{qg}",
                        name=f"attnp{hp}_{qg}",
                    )
                    for qg in range(4)
                ]
                for hp in range(2)
            ]

            def qk_group(w_i, co, tt):
                """one [128,512] tile of qT (w_i=0) or kT (w_i=1), chunk co"""
                wsb = wq_sb if w_i == 0 else wk_sb
                bias_sb = bqt_sb if w_i == 0 else bkt_sb
                qp = ps_misc.tile([128, 512], F32, tag="sm", name="qp")
                for ci in range(8):
                    nc.tensor.matmul(
                        qp[:],
                        wsb[:, ci, ts(co, 128)],
                        xT[:, ci, ts(tt, 512)],
                        start=(ci == 0),
                        stop=(ci == 7),
                    )
                nc.scalar.add(
                    qkT[:, 2 * w_i + co, ts(tt, 512)],
                    qp[:],
                    bias_sb[:, co : co + 1],
                )

            def v_group(tt):
                vp = ps_misc.tile([128, CH], F32, tag="sm", name="vp")
                for ci in range(8):
                    nc.tensor.matmul(
                        vp[:],
                        xT[:, ci, ts(tt, 128)],
                        wv_sb[:, ci, :],
                        start=(ci == 0),
                        stop=(ci == 7),
                    )
                nc.vector.tensor_add(
                    vext[:, tt, :, :],
                    vp[:].rearrange("p (h d) -> p h d", h=HG),
                    bv_rep[:].rearrange("p (h d) -> p h d", h=HG),
                )

            def emit_bias_reps():
                # bias replication along partitions via K=1 matmuls (fp32)
                bp = ps_misc.tile([128, 512], F32, tag="sm", name="bp")
                nc.tensor.matmul(
                    bp[:, 0:CH], ones_f32[0:1, :], bv_sb[0:1, :],
                    start=True, stop=True,
                )
                nc.vector.tensor_copy(bv_rep[:], bp[:, 0:CH])

            # chunk-0 q/k tiles + V first (unblocks attention round 0)
            for tt in range(4):
                qk_group(0, 0, tt)
                qk_group(1, 0, tt)

            # remaining qk groups, fed into the attention round stream below
            pending_qk = [
                (w_i, 1, tt) for tt in range(4) for w_i in (0, 1)
            ]

            p_tiles = {}

            def st_part(qg, hp):
                """scores^T + exp for head pair hp, query group qg."""
                qs = ts(qg, 512)
                pa = osb.tile([128, 8, 1024], BF, tag="p", bufs=3, name="pa")
                pb = osb.tile([128, 8, 1024], BF, tag="p", bufs=3, name="pb")
                p_tiles[(qg, hp)] = (pa, pb)
                for kp in range(8):
                    stA = ps_st.tile([128, 1024], F32, tag="st", name="stA")
                    stB = ps_st.tile([128, 1024], F32, tag="st", name="stB")
                    for j in range(2):
                        ki = 2 * kp + j
                        nc.tensor.matmul(
                            stA[:, ts(j, 512)],
                            qkT[0:64, 2 + hp, ts(ki, 128)],
                            qkT[0:64, hp, qs],
                            start=True, stop=True,
                        )
                        nc.tensor.matmul(
                            stB[:, ts(j, 512)],
                            qkT[64:128, 2 + hp, ts(ki, 128)],
                            qkT[64:128, hp, qs],
                            start=True, stop=True,
                        )
                    nc.scalar.activation(
                        pa[:, kp, :], stA[:],
                        mybir.ActivationFunctionType.Exp, scale=1.0 / 8.0,
                    )
                    nc.scalar.activation(
                        pb[:, kp, :], stB[:],
                        mybir.ActivationFunctionType.Exp, scale=1.0 / 8.0,
                    )

            def pv_part(qg, hp):
                pa, pb = p_tiles.pop((qg, hp))
                # paired PV: head 2hp -> psum partitions 0-63 (col group 0-1),
                # head 2hp+1 -> partitions 64-127 (col group 2-3); the two
                # column-tiled matmul streams run concurrently on the PE.
                pv = ps_pv.tile([128, 512], F32, tag="pv", name="pv")
                for ki in range(16):
                    for hh, pbuf in ((0, pa), (1, pb)):
                        h = 2 * hp + hh
                        nc.tensor.matmul(
                            pv[64 * hh : 64 * hh + 64, :],
                            vext[:, ki, h, :],
                            pbuf[:, ki // 2, ts(ki % 2, 512)],
                            start=(ki == 0),
                            stop=(ki == 15),
                        )
                tmp = sbn.tile([128, 512], F32, tag="tmp", name="tmp", bufs=4)
                nc.vector.tensor_copy(tmp[:], pv[:])
                tmp_tiles[(qg, hp)] = tmp
                # denominators: bf16 add-tree over the 8 kp slots (DVE 2x),
                # then a K=128 ones-matmul folds the partition axis.
                for hh, pbuf in ((0, pa), (1, pb)):
                    h = 2 * hp + hh
                    t1 = sbn.tile([128, 4, 1024], BF, tag="t1", name="t1", bufs=2)
                    nc.gpsimd.tensor_add(
                        t1[:], pbuf[:, 0:4, :], pbuf[:, 4:8, :]
                    )
                    t2 = sbn.tile([128, 2, 1024], BF, tag="t2", name="t2", bufs=2)
                    nc.vector.tensor_add(
                        t2[:], t1[:, 0:2, :], t1[:, 2:4, :]
                    )
                    t3 = sbn.tile([128, 1024], BF, tag="t3", name="t3", bufs=2)
                    nc.vector.tensor_add(
                        t3[:], t2[:, 0, :], t2[:, 1, :]
                    )
                    t4 = sbn.tile([128, 512], BF, tag="t4", name="t4", bufs=2)
                    nc.vector.tensor_add(
                        t4[:], t3[:, 0:512], t3[:, 512:1024]
                    )
                    dps = ps_misc.tile([128, 512], F32, tag="sm", name="dps")
                    nc.tensor.matmul(
                        dps[0:1, :], ones_col[:, 0:1], t4[:],
                        start=True, stop=True,
                    )
                    # reciprocal chain (DVE-only; ready well before the
                    # deferred rep-matmul reads it)
                    rec32 = sbn.tile([1, 512], F32, tag="rec32", name="rc", bufs=4)
                    nc.vector.tensor_copy(rec32[:], dps[0:1, :])
                    nc.vector.reciprocal_approx_fast(out=rec32[:], in_=rec32[:])
                    rec_bf = sbn.tile([1, 512], BF, tag="rec", name="rb")
                    nc.vector.tensor_copy(rec_bf[:], rec32[:])
                    rec_tiles[4 * qg + h] = rec_bf

            rec_tiles = {}
            tmp_tiles = {}

            def normalize_round(qg, hp):
                """rep-matmul + multiply -> attn_p[hp][qg] (both heads)."""
                rp = ps_misc.tile([128, 512], F32, tag="sm", name="rp")
                tmp = tmp_tiles.pop((qg, hp))
                for hh in range(2):
                    slot = 4 * qg + 2 * hp + hh
                    rows = slice(64 * hh, 64 * hh + 64)
                    nc.tensor.matmul(
                        rp[rows, :], ones_bf[0:1, 0:64], rec_tiles[slot][:],
                        start=True, stop=True,
                    )
                    nc.vector.tensor_mul(
                        attn_p[hp][qg][rows, :],
                        tmp[rows, :],
                        rp[rows, :],
                    )

            def outproj_chunk(qg):
                """partial out-projection rows for query group qg."""
                for tt4 in range(4):
                    tt = 4 * qg + tt4
                    o_sb = osb.tile([128, C], F32, tag="o", name="osb")
                    for cn in range(2):
                        op = ps_misc.tile(
                            [128, 512], F32, tag="sm", name="op"
                        )
                        for hp in range(2):
                            nc.tensor.matmul(
                                op[:],
                                attn_p[hp][qg][:, ts(tt4, 128)],
                                wout_sb[:, hp, ts(cn, 512)],
                                start=(hp == 0),
                                stop=(hp == 1),
                            )
                        nc.vector.tensor_copy(o_sb[:, ts(cn, 512)], op[:])
                    nc.gpsimd.dma_start(out=out[ts(tt, 128), :], in_=o_sb[:])

            # ---- pipelined main stream ----
            rounds = [(qg, hp) for qg in range(4) for hp in range(2)]
            st_part(*rounds[0])  # scores for round 0 feed ACT immediately
            emit_bias_reps()
            for tt in range(16):
                v_group(tt)
            while pending_qk:
                qk_group(*pending_qk.pop(0))
            st_part(*rounds[1])
            for r, (qg, hp) in enumerate(rounds):
                pv_part(qg, hp)
                if r + 2 < len(rounds):
                    st_part(*rounds[r + 2])
                if r >= 1:
                    pqg, php = rounds[r - 1]
                    normalize_round(pqg, php)
                    if php == 1:
                        outproj_chunk(pqg)
            normalize_round(*rounds[-1])
            outproj_chunk(rounds[-1][0])

    nc.compile()
    return nc


def _get_nc():
    global _NC_CACHE
    if _NC_CACHE is None:
        _NC_CACHE = _build_nc()
    return _NC_CACHE


def kernel(x, w_qkv, b_qkv, w_out, b_out):
    global LAST_RESULT
    x = np.asarray(x, dtype=np.float32)
    w_qkv = np.asarray(w_qkv, dtype=np.float32)
    b_qkv = np.asarray(b_qkv, dtype=np.float32)
    w_out = np.asarray(w_out, dtype=np.float32)
    b_out = np.asarray(b_out, dtype=np.float32)

    bf = ml_dtypes.bfloat16
    in_maps = []
    for c in range(N_CORES):
        b, g = divmod(c, 4)
        cols = slice(CH * g, CH * (g + 1))
        bq = b_qkv[0 * C + CH * g : 0 * C + CH * (g + 1)]
        bk = b_qkv[1 * C + CH * g : 1 * C + CH * (g + 1)]
        bvv = b_qkv[2 * C + CH * g : 2 * C + CH * (g + 1)]
        in_maps.append(
            {
                "xt": np.ascontiguousarray(x[b].astype(bf).T),
                "wq": np.ascontiguousarray(w_qkv[:, 0 * C :][:, cols]).astype(bf),
                "wk": np.ascontiguousarray(w_qkv[:, 1 * C :][:, cols]).astype(bf),
                "wv": np.ascontiguousarray(w_qkv[:, 2 * C :][:, cols]).astype(bf),
                "bqt": np.ascontiguousarray(bq.reshape(2, 128).T),
                "bkt": np.ascontiguousarray(bk.reshape(2, 128).T),
                "bv": np.ascontiguousarray(bvv.reshape(1, CH)),
                "wout": np.ascontiguousarray(w_out[CH * g : CH * (g + 1), :]).astype(bf),
            }
        )

    nc = _get_nc()
    LAST_RESULT = bass_utils.run_bass_kernel_spmd(
        nc, in_maps, core_ids=list(range(N_CORES))
    )

    full = np.zeros((B, T, C), dtype=np.float32)
    full += b_out  # broadcast bias once; cores emit pure partials
    for c in range(N_CORES):
        b = c // 4
        full[b] += LAST_RESULT.results[c]["out"]
    return full


# revision 27
# speedup vs baseline: 1.6033x; 1.6033x over previous
"""Multi-head self-attention on 8 Trainium2 NeuronCores (Bass/Tile).

Problem: x[2,2048,1024] -> MHA(16 heads, d_head 64) -> out[2,2048,1024].

Sharding (batch x head-group, Megatron-ish, collective-free):
  core c (0..7): batch b = c//4, head group g = c%4 (heads 4g..4g+3).
  Each core computes q/k/v projections for its 4 heads over its batch,
  attention for those heads, and a PARTIAL output projection
  attn_local[256ch] @ w_out[256ch rows] over the full sequence. The host
  sums the 4 partials per batch (the Megatron row-parallel all-reduce is
  folded into the unshard step; b_out/4 is added on each core so the sum
  carries the bias exactly).

On-core layout (TensorE compute in bf16, fp32 PSUM accumulation):
  - x arrives pre-transposed (x^T, bf16) from the host shard-prep, so the
    kernel does no transposes at all (the xbar serializes against every
    other DMA and was the startup bottleneck when used).
  - qT/kT in [channel, t] layout (weight-stationary matmuls): scores^T =
    kT.T @ qT needs no transposes, and the two heads of a 128-channel chunk
    sit in partitions 0-63/64-127 so their K=64 score matmuls run
    concurrently in disjoint PE row groups.
  - softmax: scores^T [128ki, qi] tiles -> ACT exp (PSUM->SBUF bf16,
    scale=1/8 folded, no max subtraction: |s|/8 <= ~2).
  - PV: attn^T = Vext.T @ P~ with Vext = [V | ones] (M=65): the ones column
    accumulates the softmax denominators in partition 64 for free.
  - normalize: reciprocal_approx_fast + K=1 ones-matmul partition-broadcast,
    software-pipelined one round behind PV so the PE queue never stalls on
    the DVE chain; out-projection chunks follow per query-group.
"""

import numpy as np
import ml_dtypes

import concourse.bass as bass
import concourse.mybir as mybir
import concourse.tile as tile
from concourse import bacc
from concourse import bass_utils
from concourse.bass import ts

BF = mybir.dt.bfloat16
F32 = mybir.dt.float32

B, T, C = 2, 2048, 1024
H, DH = 16, 64
N_CORES = 8
HG = 4  # heads per core
CH = HG * DH  # 256 channels per core

LAST_RESULT = None  # BassKernelResults of the most recent run (for profiling)
_NC_CACHE = None


def _build_nc():
    nc = bacc.Bacc(
        "TRN2", target_bir_lowering=False, debug=False, num_devices=N_CORES
    )

    xt = nc.dram_tensor("xt", [C, T], BF, kind="ExternalInput")
    wq = nc.dram_tensor("wq", [C, CH], BF, kind="ExternalInput")
    wk = nc.dram_tensor("wk", [C, CH], BF, kind="ExternalInput")
    wv = nc.dram_tensor("wv", [C, CH], BF, kind="ExternalInput")
    bqt = nc.dram_tensor("bqt", [128, 2], F32, kind="ExternalInput")
    bkt = nc.dram_tensor("bkt", [128, 2], F32, kind="ExternalInput")
    bv = nc.dram_tensor("bv", [1, CH], F32, kind="ExternalInput")
    wout = nc.dram_tensor("wout", [CH, C], BF, kind="ExternalInput")
    out = nc.dram_tensor("out", [T, C], F32, kind="ExternalOutput")

    with tile.TileContext(nc) as tc:
        with (
            tc.tile_pool(name="persist", bufs=1) as persist,
            tc.tile_pool(name="consts", bufs=1) as consts,
            tc.tile_pool(name="sbn", bufs=6) as sbn,
            tc.tile_pool(name="osb", bufs=3) as osb,
            tc.tile_pool(name="ps_st", bufs=2, space="PSUM") as ps_st,
            tc.tile_pool(name="ps_pv", bufs=2, space="PSUM") as ps_pv,
            tc.tile_pool(name="ps_misc", bufs=2, space="PSUM") as ps_misc,
        ):
            # ---- x^T + weights, one HWDGE queue, dependency-ordered ----
            ones_bf = consts.tile([1, 128], BF)
            nc.vector.memset(ones_bf[:], 1.0)
            ones_col = consts.tile([128, 1], BF)
            nc.vector.memset(ones_col[:], 1.0)
            ones_f32 = consts.tile([1, 128], F32)
            nc.vector.memset(ones_f32[:], 1.0)

            xT = persist.tile([128, 8, T], BF, tag="xT")
            wq_sb = persist.tile([128, 8, CH], BF, tag="wq")
            wk_sb = persist.tile([128, 8, CH], BF, tag="wk")
            wv_sb = persist.tile([128, 8, CH], BF, tag="wv")
            wout_sb = persist.tile([128, 2, C], BF, tag="wout")
            bqt_sb = consts.tile([128, 2], F32)
            bkt_sb = consts.tile([128, 2], F32)
            bv_sb = consts.tile([1, CH], F32)

            xt_r = xt.rearrange("(ci p) t -> p ci t", p=128)
            nc.sync.dma_start(
                out=wq_sb[:], in_=wq.rearrange("(ci p) j -> p ci j", p=128)
            )
            nc.sync.dma_start(
                out=wk_sb[:], in_=wk.rearrange("(ci p) j -> p ci j", p=128)
            )
            nc.sync.dma_start(out=xT[:, 0, :], in_=xt_r[:, 0, :])
            nc.sync.dma_start(out=bqt_sb[:], in_=bqt[:])
            nc.sync.dma_start(out=bkt_sb[:], in_=bkt[:])
            nc.sync.dma_start(out=xT[:, 1, :], in_=xt_r[:, 1, :])
            nc.sync.dma_start(
                out=wv_sb[:], in_=wv.rearrange("(ci p) j -> p ci j", p=128)
            )
            nc.sync.dma_start(out=bv_sb[:], in_=bv[:])
            for ci in range(2, 8):
                nc.sync.dma_start(out=xT[:, ci, :], in_=xt_r[:, ci, :])
            nc.sync.dma_start(
                out=wout_sb[:], in_=wout.rearrange("(hp p) j -> p hp j", p=128)
            )

            bv_rep = persist.tile([128, CH], F32, tag="bv_rep")

            # ---- persistent activations ----
            # qkT[:, 0:2, :] = qT chunks (hp), [:, 2:4, :] = kT chunks;
            # chunk hp rows 0-63 = head 2hp, rows 64-127 = head 2hp+1.
            qkT = persist.tile([128, 4, T], BF, tag="qkT")
            vext = persist.tile([128, T // 128, HG, DH], BF, tag="vext")
            attn_p = [
                [
                    persist.tile(
                        [128, 512], BF, tag=f"attnp{hp}_{qg}",
                        name=f"attnp{hp}_{qg}",
                    )
                    for qg in range(4)
                ]
                for hp in range(2)
            ]

            def qk_group(w_i, co, tt):
                """one [128,512] tile of qT (w_i=0) or kT (w_i=1), chunk co"""
                wsb = wq_sb if w_i == 0 else wk_sb
                bias_sb = bqt_sb if w_i == 0 else bkt_sb
                qp = ps_misc.tile([128, 512], F32, tag="sm", name="qp")
                for ci in range(8):
                    nc.tensor.matmul(
                        qp[:],
                        wsb[:, ci, ts(co, 128)],
                        xT[:, ci, ts(tt, 512)],
                        start=(ci == 0),
                        stop=(ci == 7),
                    )
                nc.scalar.add(
                    qkT[:, 2 * w_i + co, ts(tt, 512)],
                    qp[:],
                    bias_sb[:, co : co + 1],
                )

            def v_group(tt):
                vp = ps_misc.tile([128, CH], F32, tag="sm", name="vp")
                for ci in range(8):
                    nc.tensor.matmul(
                        vp[:],
                        xT[:, ci, ts(tt, 128)],
                        wv_sb[:, ci, :],
                        start=(ci == 0),
                        stop=(ci == 7),
                    )
                nc.vector.tensor_add(
                    vext[:, tt, :, :],
                    vp[:].rearrange("p (h d) -> p h d", h=HG),
                    bv_rep[:].rearrange("p (h d) -> p h d", h=HG),
                )

            def emit_bias_reps():
                # bias replication along partitions via K=1 matmuls (fp32)
                bp = ps_misc.tile([128, 512], F32, tag="sm", name="bp")
                nc.tensor.matmul(
                    bp[:, 0:CH], ones_f32[0:1, :], bv_sb[0:1, :],
                    start=True, stop=True,
                )
                nc.vector.tensor_copy(bv_rep[:], bp[:, 0:CH])

            # chunk-0 q/k tiles + V first (unblocks attention round 0)
            for tt in range(4):
                qk_group(0, 0, tt)
                qk_group(1, 0, tt)

            # remaining qk groups, fed into the attention round stream below
            pending_qk = [
                (w_i, 1, tt) for tt in range(4) for w_i in (0, 1)
            ]

            p_tiles = {}

            def st_part(qg, hp):
                """scores^T + exp for head pair hp, query group qg."""
                qs = ts(qg, 512)
                pa = osb.tile([128, 8, 1024], BF, tag="p", bufs=3, name="pa")
                pb = osb.tile([128, 8, 1024], BF, tag="p", bufs=3, name="pb")
                p_tiles[(qg, hp)] = (pa, pb)
                for kp in range(8):
                    stA = ps_st.tile([128, 1024], F32, tag="st", name="stA")
                    stB = ps_st.tile([128, 1024], F32, tag="st", name="stB")
                    for j in range(2):
                        ki = 2 * kp + j
                        nc.tensor.matmul(
                            stA[:, ts(j, 512)],
                            qkT[0:64, 2 + hp, ts(ki, 128)],
                            qkT[0:64, hp, qs],
                            start=True, stop=True,
                        )
                        nc.tensor.matmul(
                            stB[:, ts(j, 512)],
                            qkT[64:128, 2 + hp, ts(ki, 128)],
                            qkT[64:128, hp, qs],
                            start=True, stop=True,
                        )
                    nc.scalar.activation(
                        pa[:, kp, :], stA[:],
                        mybir.ActivationFunctionType.Exp, scale=1.0 / 8.0,
                    )
                    nc.scalar.activation(
                        pb[:, kp, :], stB[:],
                        mybir.ActivationFunctionType.Exp, scale=1.0 / 8.0,
                    )

            def pv_part(qg, hp):
                pa, pb = p_tiles.pop((qg, hp))
                # paired PV: head 2hp -> psum partitions 0-63 (col group 0-1),
                # head 2hp+1 -> partitions 64-127 (col group 2-3); the two
                # column-tiled matmul streams run concurrently on the PE.
                pv = ps_pv.tile([128, 512], F32, tag="pv", name="pv")
                for ki in range(16):
                    for hh, pbuf in ((0, pa), (1, pb)):
                        h = 2 * hp + hh
                        nc.tensor.matmul(
                            pv[64 * hh : 64 * hh + 64, :],
                            vext[:, ki, h, :],
                            pbuf[:, ki // 2, ts(ki % 2, 512)],
                            start=(ki == 0),
                            stop=(ki == 15),
                        )
                tmp = sbn.tile([128, 512], F32, tag="tmp", name="tmp", bufs=4)
                nc.vector.tensor_copy(tmp[:], pv[:])
                tmp_tiles[(qg, hp)] = tmp
                # denominators: bf16 add-tree over the 8 kp slots (DVE 2x),
                # then a K=128 ones-matmul folds the partition axis.
                for hh, pbuf in ((0, pa), (1, pb)):
                    h = 2 * hp + hh
                    t1 = sbn.tile([128, 4, 1024], BF, tag="t1", name="t1", bufs=2)
                    nc.vector.tensor_add(
                        t1[:], pbuf[:, 0:4, :], pbuf[:, 4:8, :]
                    )
                    t2 = sbn.tile([128, 2, 1024], BF, tag="t2", name="t2", bufs=2)
                    nc.vector.tensor_add(
                        t2[:], t1[:, 0:2, :], t1[:, 2:4, :]
                    )
                    t3 = sbn.tile([128, 1024], BF, tag="t3", name="t3", bufs=2)
                    nc.vector.tensor_add(
                        t3[:], t2[:, 0, :], t2[:, 1, :]
                    )
                    t4 = sbn.tile([128, 512], BF, tag="t4", name="t4", bufs=2)
                    nc.vector.tensor_add(
                        t4[:], t3[:, 0:512], t3[:, 512:1024]
                    )
                    dps = ps_misc.tile([128, 512], F32, tag="sm", name="dps")
                    nc.tensor.matmul(
                        dps[0:1, :], ones_col[:, 0:1], t4[:],
                        start=True, stop=True,
                    )
                    # reciprocal chain (DVE-only; ready well before the
                    # deferred rep-matmul reads it)
                    rec32 = sbn.tile([1, 512], F32, tag="rec32", name="rc", bufs=4)
                    nc.vector.tensor_copy(rec32[:], dps[0:1, :])
                    nc.vector.reciprocal_approx_fast(out=rec32[:], in_=rec32[:])
                    rec_bf = sbn.tile([1, 512], BF, tag="rec", name="rb")
                    nc.vector.tensor_copy(rec_bf[:], rec32[:])
                    rec_tiles[4 * qg + h] = rec_bf

            rec_tiles = {}
            tmp_tiles = {}

            def normalize_round(qg, hp):
                """rep-matmul + multiply -> attn_p[hp][qg] (both heads)."""
                rp = ps_misc.tile([128, 512], F32, tag="sm", name="rp")
                tmp = tmp_tiles.pop((qg, hp))
                for hh in range(2):
                    slot = 4 * qg + 2 * hp + hh
                    rows = slice(64 * hh, 64 * hh + 64)
                    nc.tensor.matmul(
                        rp[rows, :], ones_bf[0:1, 0:64], rec_tiles[slot][:],
                        start=True, stop=True,
                    )
                    nc.vector.tensor_mul(
                        attn_p[hp][qg][rows, :],
                        tmp[rows, :],
                        rp[rows, :],
                    )

            def outproj_chunk(qg):
                """partial out-projection rows for query group qg."""
                for tt4 in range(4):
                    tt = 4 * qg + tt4
                    o_sb = osb.tile([128, C], F32, tag="o", name="osb")
                    for cn in range(2):
                        op = ps_misc.tile(
                            [128, 512], F32, tag="sm", name="op"
                        )
                        for hp in range(2):
                            nc.tensor.matmul(
                                op[:],
                                attn_p[hp][qg][:, ts(tt4, 128)],
                                wout_sb[:, hp, ts(cn, 512)],
                                start=(hp == 0),
                                stop=(hp == 1),
                            )
                        nc.vector.tensor_copy(o_sb[:, ts(cn, 512)], op[:])
                    nc.gpsimd.dma_start(out=out[ts(tt, 128), :], in_=o_sb[:])

            # ---- pipelined main stream ----
            rounds = [(qg, hp) for qg in range(4) for hp in range(2)]
            st_part(*rounds[0])  # scores for round 0 feed ACT immediately
            emit_bias_reps()
            for tt in range(16):
                v_group(tt)
            while pending_qk:
                qk_group(*pending_qk.pop(0))
            st_part(*rounds[1])
            for r, (qg, hp) in enumerate(rounds):
                pv_part(qg, hp)
                if r + 2 < len(rounds):
                    st_part(*rounds[r + 2])
                if r >= 1:
                    pqg, php = rounds[r - 1]
                    normalize_round(pqg, php)
                    if php == 1:
                        outproj_chunk(pqg)
            normalize_round(*rounds[-1])
            outproj_chunk(rounds[-1][0])

    nc.compile()
    return nc


def _get_nc():
    global _NC_CACHE
    if _NC_CACHE is None:
        _NC_CACHE = _build_nc()
    return _NC_CACHE


def kernel(x, w_qkv, b_qkv, w_out, b_out):
    global LAST_RESULT
    x = np.asarray(x, dtype=np.float32)
    w_qkv = np.asarray(w_qkv, dtype=np.float32)
    b_qkv = np.asarray(b_qkv, dtype=np.float32)
    w_out = np.asarray(w_out, dtype=np.float32)
    b_out = np.asarray(b_out, dtype=np.float32)

    bf = ml_dtypes.bfloat16
    in_maps = []
    for c in range(N_CORES):
        b, g = divmod(c, 4)
        cols = slice(CH * g, CH * (g + 1))
        bq = b_qkv[0 * C + CH * g : 0 * C + CH * (g + 1)]
        bk = b_qkv[1 * C + CH * g : 1 * C + CH * (g + 1)]
        bvv = b_qkv[2 * C + CH * g : 2 * C + CH * (g + 1)]
        in_maps.append(
            {
                "xt": np.ascontiguousarray(x[b].astype(bf).T),
                "wq": np.ascontiguousarray(w_qkv[:, 0 * C :][:, cols]).astype(bf),
                "wk": np.ascontiguousarray(w_qkv[:, 1 * C :][:, cols]).astype(bf),
                "wv": np.ascontiguousarray(w_qkv[:, 2 * C :][:, cols]).astype(bf),
                "bqt": np.ascontiguousarray(bq.reshape(2, 128).T),
                "bkt": np.ascontiguousarray(bk.reshape(2, 128).T),
                "bv": np.ascontiguousarray(bvv.reshape(1, CH)),
                "wout": np.ascontiguousarray(w_out[CH * g : CH * (g + 1), :]).astype(bf),
            }
        )

    nc = _get_nc()
    LAST_RESULT = bass_utils.run_bass_kernel_spmd(
        nc, in_maps, core_ids=list(range(N_CORES))
    )

    full = np.zeros((B, T, C), dtype=np.float32)
    full += b_out  # broadcast bias once; cores emit pure partials
    for c in range(N_CORES):
        b = c // 4
        full[b] += LAST_RESULT.results[c]["out"]
    return full
